# revision 20
# baseline (speedup 1.0000x reference)
"""Trainium2 Bass kernel for nn_CrossAttentionBridge.

Math: with KV length 1, softmax over the single key is exactly 1.0, so the
attention output is V broadcast over T and the whole module collapses to

    out = last_hidden + sigmoid(gate) * ((zH @ W_mem) @ W_v @ W_o)   (per batch)

The Q/K GEMMs are dead compute (attention weights are identically 1.0
regardless of Q), and the O GEMM collapses to a single vector-matrix chain
per batch.  What remains on device:

  Launch 1 ("chain", 8 cores): the tiny vector-matrix chain
      vo[b, :] = (zH[b] @ W_mem) @ W_v @ W_o        (4 x 2048 result)
    sharded over the 2048-wide inner contraction: core i holds
    W_v[:, ci] and W_o[ci, :] (256 columns/rows each) and computes a
    partial vo; the host sums the 8 partials (exact - the chain is linear).

  Launch 2 ("main", 8 cores): the 256 MB broadcast-add
      Y = X + sigmoid(gate) * vo[b]
    data-parallel over (B, T): core i takes a contiguous (2048, 2048)
    token slice of one batch (2 cores per batch), with vo[b] replicated
    across 128 partitions host-side and sigmoid applied on device.
"""

import os
import sys

for _p in ("/opt/trn_rl_repo", "/root/.axon_site/_ro/trn_rl_repo"):
    if os.path.isdir(_p) and _p not in sys.path:
        sys.path.insert(0, _p)

from contextlib import ExitStack

import numpy as np

import concourse.bacc as bacc
import concourse.bass as bass
import concourse.mybir as mybir
import concourse.tile as tile
from concourse.bass_utils import run_bass_kernel_spmd

B, T, D, DH = 4, 4096, 2048, 512
N_CORES = 8
ROWS = B * T // N_CORES          # 2048 token rows per core in the main launch
KS = D // N_CORES                # 256-wide contraction slice in the chain launch
F32 = mybir.dt.float32
CHUNK = 2                        # row-tiles per DMA in the main launch (2 MiB)
N_TILES = ROWS // (128 * CHUNK)  # outer iterations in the main launch

# Per-launch HW exec times (ns) from the last traced run, for test harnesses.
LAST_EXEC_NS = {}


def build_chain_nc(use_f32r=True):
    """vo_part[b, :] = sigmoid(gate) * ((zH[b] @ W_mem) @ W_v[:, ci]) @ W_o[ci, :].

    All weights arrive host-packed as (128, k, free) so every DMA is
    contiguous per partition.  Matmuls are oriented transpose-free:
        memT = W_mem^T @ zH^T   -> lhsT = W_mem chunk,  rhs = zH^T chunk
        VT   = Wv^T @ memT      -> lhsT = Wv chunk,     rhs = memT chunk
        vo   = VT^T @ Wo        -> lhsT = VT chunk,     rhs = Wo chunk
    Weight DMAs are split per contraction chunk so PE work streams behind
    the loads instead of waiting for whole tensors.
    """
    F32R = mybir.dt.float32r if use_f32r else F32
    nc = bacc.Bacc("TRN2", target_bir_lowering=False, debug=False)
    zt_d = nc.dram_tensor("zT", [128, 4, B], F32, kind="ExternalInput").ap()
    g_d = nc.dram_tensor("g", [B, 1], F32, kind="ExternalInput").ap()
    wm_d = nc.dram_tensor("Wm", [128, 4, D], F32, kind="ExternalInput").ap()
    wv_d = nc.dram_tensor("Wv", [128, 16, KS], F32, kind="ExternalInput").ap()
    wo_d = nc.dram_tensor("Wo", [128, 2, D], F32R, kind="ExternalInput").ap()
    vo_d = nc.dram_tensor("vo", [B, D], F32, kind="ExternalOutput").ap()

    with tile.TileContext(nc) as tc, ExitStack() as ctx:
        pool = ctx.enter_context(tc.tile_pool(name="sb", bufs=1))
        psum = ctx.enter_context(tc.tile_pool(name="ps", bufs=2, space="PSUM"))
        psum_o = ctx.enter_context(tc.tile_pool(name="pso", bufs=1, space="PSUM"))

        zt = pool.tile([128, 4, B], F32)
        nc.sync.dma_start(zt[:], zt_d[:])
        g_t = pool.tile([B, 1], F32)
        nc.sync.dma_start(g_t[:], g_d[:])
        sg_t = pool.tile([B, 1], F32)
        nc.scalar.activation(sg_t[:], g_t[:], mybir.ActivationFunctionType.Sigmoid)

        # Weight loads split into ~0.5 MB pieces and greedily balanced over
        # the three DMA-issue engines (SP HWDGE, ACT HWDGE, POOL SWDGE) so
        # the rings stream in parallel and stage A can start early.
        wm = pool.tile([128, 4, D], F32)
        wv = pool.tile([128, 16, KS], F32)
        wo = pool.tile([128, 2, D], F32R)
        jobs = (
            [(wm[:, k, bass.ts(h, 1024)], wm_d[:, k, bass.ts(h, 1024)])
             for k in range(4) for h in range(2)]
            + [(wv[:, bass.ts(k4, 4), :], wv_d[:, bass.ts(k4, 4), :]) for k4 in range(4)]
            + [(wo[:, kk, bass.ts(h, 1024)], wo_d[:, kk, bass.ts(h, 1024)])
               for kk in range(2) for h in range(2)]
        )
        engs = [nc.sync, nc.scalar, nc.gpsimd]
        for j, (dst, src) in enumerate(jobs):
            engs[j % 3].dma_start(dst, src)

        # Stage A: memT (2048, 4) as (128, 16, 4) SBUF tile.  k inner so the
        # m-th accumulation only depends on wm chunks as they stream in.
        memT = pool.tile([128, 16, 4], F32)
        for m in range(16):
            pt = psum.tile([128, 4], F32, tag="acc")
            for k in range(4):
                nc.tensor.matmul(
                    pt[:],
                    lhsT=wm[:, k, bass.ts(m, 128)],
                    rhs=zt[:, k, :],
                    start=(k == 0),
                    stop=(k == 3),
                )
            nc.vector.tensor_copy(memT[:, m, :], pt[:])

        # Stage B: VT (256, 4) as (128, 2, 4) SBUF tile.  Consumes memT
        # chunk k right after stage A's m=k iteration produced it.  vt is
        # written as float32r by the DVE copy (a rounding producer), which
        # the BIR verifier requires for fp32r matmul operands.
        vt = pool.tile([128, 2, 4], F32R)
        for mm in range(2):
            pv = psum.tile([128, 4], F32, tag="acc")
            for k in range(16):
                nc.tensor.matmul(
                    pv[:],
                    lhsT=wv[:, k, bass.ts(mm, 128)],
                    rhs=memT[:, k, :],
                    start=(k == 0),
                    stop=(k == 15),
                )
            nc.vector.tensor_copy(vt[:, mm, :], pv[:])

        # Stage C: vo (4, 2048) = VT^T @ Wo, scaled by sigmoid(gate).
        # float32r runs the PE at 1 cycle/row for N>=256 (vs 4 for fp32) —
        # these 8 N=512 matmuls are the chain's serial tail after the DMAs.
        po = psum_o.tile([4, D], F32, tag="po")
        for n in range(4):
            for kk in range(2):
                nc.tensor.matmul(
                    po[:, bass.ts(n, 512)],
                    lhsT=vt[:, kk, :],
                    rhs=wo[:, kk, bass.ts(n, 512)],
                    start=(kk == 0),
                    stop=(kk == 1),
                )
        vo_sb = pool.tile([B, D], F32)
        nc.vector.tensor_scalar_mul(vo_sb[:], po[:], sg_t[:])
        nc.sync.dma_start(vo_d[:], vo_sb[:])

    nc.compile()
    return nc


def build_main_nc(chunk=CHUNK, bufs=4):
    """Y = X + gvo_rep (vo arrives pre-scaled by sigmoid(gate))."""
    nc = bacc.Bacc("TRN2", target_bir_lowering=False, debug=False)
    x_d = nc.dram_tensor("X", [ROWS, D], F32, kind="ExternalInput").ap()
    vo_d = nc.dram_tensor("vo", [128, D], F32, kind="ExternalInput").ap()
    y_d = nc.dram_tensor("Y", [ROWS, D], F32, kind="ExternalOutput").ap()

    xr = x_d.rearrange("(n c p) d -> n p c d", p=128, c=chunk)
    yr = y_d.rearrange("(n c p) d -> n p c d", p=128, c=chunk)

    with tile.TileContext(nc) as tc, ExitStack() as ctx:
        cpool = ctx.enter_context(tc.tile_pool(name="const", bufs=1))
        xpool = ctx.enter_context(tc.tile_pool(name="x", bufs=bufs))

        vo_t = cpool.tile([128, D], F32)
        nc.sync.dma_start(vo_t[:], vo_d[:])

        # One 1 MiB DMA per 128-row group, round-robin over the three
        # DMA-issue engines (SP HWDGE / ACT HWDGE / POOL SWDGE) so the DGE
        # rings stream concurrently.  DVE stays free for the adds.
        engs = [nc.sync, nc.scalar, nc.gpsimd]
        j = 0
        for i in range(ROWS // (128 * chunk)):
            xt = xpool.tile([128, chunk, D], F32, tag="xt")
            for c in range(chunk):
                engs[j % 3].dma_start(xt[:, c, :], xr[i][:, c, :])
                j += 1
            for c in range(chunk):
                nc.vector.tensor_add(xt[:, c, :], xt[:, c, :], vo_t[:])
            for c in range(chunk):
                engs[j % 3].dma_start(yr[i][:, c, :], xt[:, c, :])
                j += 1

    nc.compile()
    return nc


_NC_CACHE = {}


def _get_ncs():
    if "chain" not in _NC_CACHE:
        _NC_CACHE["chain"] = build_chain_nc()
        _NC_CACHE["main"] = build_main_nc()
    return _NC_CACHE["chain"], _NC_CACHE["main"]


def _pack_p128(a, k):
    """(k*128, f) -> contiguous (128, k, f)."""
    f = a.shape[1]
    return np.ascontiguousarray(a.reshape(k, 128, f).transpose(1, 0, 2))


def kernel(last_hidden, zH, W_mem, W_q, W_k, W_v, W_o, gate, trace=False):
    last_hidden = np.ascontiguousarray(np.asarray(last_hidden, dtype=np.float32))
    zH = np.asarray(zH, dtype=np.float32)
    W_mem = np.asarray(W_mem, dtype=np.float32)
    W_v = np.asarray(W_v, dtype=np.float32)
    W_o = np.asarray(W_o, dtype=np.float32)
    gate_f = float(np.asarray(gate, dtype=np.float32))

    nc_chain, nc_main = _get_ncs()
    core_ids = list(range(N_CORES))

    # ---- Launch 1: partial vo chains, contraction sliced 8 ways ----
    zT = _pack_p128(np.ascontiguousarray(zH.reshape(B, DH).T), 4)     # (128,4,4)
    Wm = _pack_p128(W_mem, 4)                                         # (128,4,2048)
    g4 = np.full((B, 1), gate_f, dtype=np.float32)
    in1 = []
    for i in core_ids:
        sl = slice(i * KS, (i + 1) * KS)
        in1.append({
            "zT": zT,
            "g": g4,
            "Wm": Wm,
            "Wv": _pack_p128(np.ascontiguousarray(W_v[:, sl]), 16),   # (128,16,256)
            "Wo": _pack_p128(np.ascontiguousarray(W_o[sl, :]), 2),    # (128,2,2048)
        })
    r1 = run_bass_kernel_spmd(nc_chain, in1, core_ids=core_ids, trace=trace)
    # Partials are already scaled by sigmoid(gate); summing is exact (linear).
    vo = np.sum([r["vo"] for r in r1.results], axis=0, dtype=np.float32)  # (4,2048)

    # ---- Launch 2: Y = X + vo[b] ----
    x_flat = last_hidden.reshape(B * T, D)
    in2 = []
    for i in core_ids:
        in2.append({
            "X": x_flat[i * ROWS:(i + 1) * ROWS],
            "vo": np.ascontiguousarray(np.broadcast_to(vo[i // 2], (128, D))),
        })
    r2 = run_bass_kernel_spmd(nc_main, in2, core_ids=core_ids, trace=trace)

    LAST_EXEC_NS["chain"] = r1.exec_time_ns
    LAST_EXEC_NS["main"] = r2.exec_time_ns

    out = np.concatenate([r["Y"] for r in r2.results], axis=0)
    return out.reshape(B, T, D)
